# revision 1
# baseline (speedup 1.0000x reference)
"""Trainium2 Bass kernel for nn_AltAttention (dense transformer attention block).

Reference computation (B=4, S=2048, D=512, H=8, Dh=64):
    qkv  = hidden @ W_qkv + b_qkv                      -> q, k, v per head
    attn = softmax(q k^T * D**-0.5 + alibi, masked)
    out  = (attn @ v) @ W_proj + b_proj

Sharding: one head per NeuronCore (8 heads = 8 cores). Each core computes
q/k/v for its head from the full hidden states, runs flash-style attention
with transposed score tiles [ks, qs] (softmax denominator folded into the
attn@V matmul via a ones column in the V operand), applies the 1/sum scaling
and the proj matmul on-chip, and writes a partial projection output. The
host sums the 8 partials (the tensor-parallel all-reduce) to form the output.

Key tricks:
  - exp(s + alibi) = exp(s) * exp(alibi): exp(alibi) is precomputed on the
    host (transposed + tiled, bf16) so the on-chip alibi add becomes a bf16
    2x-mode DVE/GPSIMD multiply instead of a 1x fp32 add.
  - scores use a duplicated-K trick: q^T and k^T are stored twice along the
    partition dim, so the Dh=64 contraction runs as K=128 (full PE array);
    the doubled sum is compensated in the host-folded weight scale.
  - softmax sums arrive free as row 0 of the attn@V output (ones column in
    the V operand); 1/sum is partition-broadcast with an SBUF->SBUF DMA and
    folded into the proj lhsT; the proj bias rides row 0 of W_proj_aug
    (only core 0 carries the bias; other cores carry zeros).
  - qkv biases are folded into the PSUM->SBUF eviction as per-partition
    tensor_scalar adds (no bias matmuls).
  - all DMA transposes (for the V operand) run back-to-back to avoid
    XBAR-mode thrash against copy-mode DMAs.
"""

import sys

sys.path.insert(0, "/opt/trn_rl_repo")

import numpy as np
import ml_dtypes

import concourse.bass as bass
import concourse.tile as tile
from concourse import bacc, mybir
from concourse.bass_utils import run_bass_kernel_spmd

BF16 = mybir.dt.bfloat16
F32 = mybir.dt.float32
NP_BF16 = ml_dtypes.bfloat16

B, S, D, H = 4, 2048, 512, 8
Dh = D // H  # 64
BS = B * S  # 8192
P = 128
NKT = S // P  # 16 ks tiles per batch
NQB = S // 512  # 4 query blocks of 512 per batch
NSC = S // 512  # 4 s-chunks of 512 per batch (qkv phase)
SCALE = D ** (-0.5)


def build_program(eb: int, repeat: int = 1, phases=(1, 2), skel=False):
    """Build the per-core Bass program. eb = number of exp-alibi slices
    (1 when the attention mask is all ones, B otherwise)."""
    nc = bacc.Bacc("TRN2", target_bir_lowering=False, debug=False, num_devices=H)

    hiddenT = nc.dram_tensor("hiddenT", [D, BS], BF16, kind="ExternalInput")
    # ea layout: [eb, NQB, 128, NKT, 512] so each (e, qb) slice is one
    # contiguous 2 MB DMA
    ea = nc.dram_tensor("ea", [eb, NQB, P, NKT, 512], BF16,
                        kind="ExternalInput")
    wqk = nc.dram_tensor("wqk", [4, P, P], BF16, kind="ExternalInput")
    bqk = nc.dram_tensor("bqk", [P, 1], F32, kind="ExternalInput")
    wv = nc.dram_tensor("wv", [4, P, Dh], BF16, kind="ExternalInput")
    bv = nc.dram_tensor("bv", [Dh, 1], F32, kind="ExternalInput")
    wproj = nc.dram_tensor("wproj", [Dh + 1, D], BF16, kind="ExternalInput")
    part = nc.dram_tensor("part", [BS, D], F32, kind="ExternalOutput")

    hT_re = hiddenT[:].rearrange("(c p) s -> p c s", p=P)  # [128, 4, 8192]

    with tile.TileContext(nc) as tc:
        with tc.tile_pool(name="consts", bufs=1) as consts, \
             tc.tile_pool(name="persist", bufs=1) as persist:
            wqk_sb = consts.tile([P, 4, P], BF16)
            nc.sync.dma_start(wqk_sb[:], wqk[:].rearrange("c p m -> p c m"))
            wv_sb = consts.tile([P, 4, Dh], BF16)
            nc.sync.dma_start(wv_sb[:], wv[:].rearrange("c p m -> p c m"))
            bqk_sb = consts.tile([P, 1], F32)
            nc.sync.dma_start(bqk_sb[:], bqk[:])
            bv_sb = consts.tile([Dh, 1], F32)
            nc.sync.dma_start(bv_sb[:], bv[:])
            wproj_sb = consts.tile([Dh + 1, D], BF16)
            nc.sync.dma_start(wproj_sb[:], wproj[:])
            ones_f32 = consts.tile([1, Dh + 1], F32)
            nc.vector.memset(ones_f32[:], 1.0)

            qT2 = persist.tile([P, BS], BF16)  # rows 0:64 qT, 64:128 qT again
            kT2 = persist.tile([P, BS], BF16)
            # padded layout: tile t = [:, t, 63:128]; col 63 = ones (sums row),
            # cols 64:128 = v^T (DMA-transpose needs 128B-aligned dest offsets)
            vaug = persist.tile([P, B * NKT, P], BF16)
            nc.vector.memset(vaug[:, :, Dh - 1 : Dh], 1.0)
            vt_all = persist.tile([Dh, B * NSC, 512], BF16)

            if skel:
                p_fix = persist.tile([P, NKT, 512], BF16)
                nc.vector.memset(p_fix[:], 0.01)
                xs_fix = persist.tile([Dh + 1, 512], BF16)
                nc.vector.memset(xs_fix[:], 0.02)
            if 1 not in phases:
                nc.vector.memset(qT2[:], 0.01)
                nc.vector.memset(kT2[:], 0.01)
                nc.vector.memset(vaug[:], 0.01)
                nc.vector.memset(vaug[:, :, Dh - 1 : Dh], 1.0)

            for rep in range(repeat):
                # ---------------- phase 1: qkv projections ----------------
                if 1 in phases:
                 with tc.tile_pool(name="hpool", bufs=2) as hpool, \
                      tc.tile_pool(name="qkps", bufs=2, space="PSUM") as qkps, \
                      tc.tile_pool(name="vtps", bufs=2, space="PSUM") as vtps:
                    for b in range(B):
                        ht = hpool.tile([P, 4, S], BF16)
                        nc.sync.dma_start(ht[:],
                                          hT_re[:, :, b * S : (b + 1) * S])
                        for sci in range(NSC):
                            col0 = b * S + sci * 512
                            ssl = slice(sci * 512, (sci + 1) * 512)

                            qk_ps = qkps.tile([P, 512], F32)
                            for c in range(4):
                                nc.tensor.matmul(qk_ps[:], wqk_sb[:, c, :],
                                                 ht[:, c, ssl],
                                                 start=(c == 0), stop=(c == 3))
                            sl = slice(col0, col0 + 512)
                            nc.vector.tensor_scalar_add(
                                qT2[0:Dh, sl], qk_ps[0:Dh, :], bqk_sb[0:Dh, :])
                            nc.vector.tensor_scalar_add(
                                kT2[Dh:P, sl], qk_ps[Dh:P, :], bqk_sb[Dh:P, :])

                            vt_ps = vtps.tile([Dh, 512], F32)
                            for c in range(4):
                                nc.tensor.matmul(vt_ps[:], wv_sb[:, c, :],
                                                 ht[:, c, ssl],
                                                 start=(c == 0), stop=(c == 3))
                            i = b * NSC + sci
                            nc.vector.tensor_scalar_add(
                                vt_all[:, i, :], vt_ps[:], bv_sb[:])
                        bsl = slice(b * S, (b + 1) * S)
                        nc.sync.dma_start(qT2[Dh:P, bsl], qT2[0:Dh, bsl])
                        nc.sync.dma_start(kT2[0:Dh, bsl], kT2[Dh:P, bsl])
                    # all transposes back-to-back: a single XBAR-mode
                    # transition on the DMA path instead of one per chunk
                    for i in range(B * NSC):
                        nc.sync.dma_start(vaug[:, i * 4 : i * 4 + 4, Dh:P],
                                          vt_all[:, i, :], transpose=True)

                # ---------------- phase 2: attention + proj ----------------
                if 2 in phases:
                 with tc.tile_pool(name="eapool", bufs=2) as eapool, \
                      tc.tile_pool(name="ppool", bufs=2) as ppool, \
                      tc.tile_pool(name="xspool", bufs=2) as xspool, \
                      tc.tile_pool(name="rsbpool", bufs=2) as rsbpool, \
                      tc.tile_pool(name="recpool", bufs=2) as recpool, \
                      tc.tile_pool(name="outpool", bufs=2) as outpool, \
                      tc.tile_pool(name="spool", bufs=2, space="PSUM") as spool, \
                      tc.tile_pool(name="xpool", bufs=2, space="PSUM") as xpool, \
                      tc.tile_pool(name="ops", bufs=2, space="PSUM") as ops:
                    for qb in range(NQB):
                        if eb == 1:
                            ea_t = eapool.tile([P, NKT, 512], BF16)
                            nc.sync.dma_start(ea_t[:], ea[0, qb])
                        for b in range(B):
                            if eb != 1:
                                ea_t = eapool.tile([P, NKT, 512], BF16)
                                nc.sync.dma_start(ea_t[:], ea[b, qb])
                            qsl = slice(b * S + qb * 512, b * S + (qb + 1) * 512)
                            x_ps = xpool.tile([Dh + 1, 512], F32)
                            p_all = ppool.tile([P, NKT, 512], BF16)
                            for g in range(NKT // 2):
                                s_ps = spool.tile([P, 1024], F32)
                                for j in range(2):
                                    tk = g * 2 + j
                                    ksl = slice(b * S + tk * P,
                                                b * S + (tk + 1) * P)
                                    nc.tensor.matmul(
                                        s_ps[:, j * 512 : (j + 1) * 512],
                                        kT2[:, ksl], qT2[:, qsl],
                                        start=True, stop=True)
                                psl = p_all[:, 2 * g : 2 * g + 2, :].rearrange(
                                    "p a b -> p (a b)")
                                easl = ea_t[:, 2 * g : 2 * g + 2, :].rearrange(
                                    "p a b -> p (a b)")
                                if not skel:
                                    nc.scalar.activation(
                                        psl, s_ps[:],
                                        mybir.ActivationFunctionType.Exp)
                                    nc.vector.tensor_mul(psl, psl, easl)
                                for j in range(2):
                                    tk = g * 2 + j
                                    t = b * NKT + tk
                                    nc.tensor.matmul(
                                        x_ps[:], vaug[:, t, Dh - 1 : P],
                                        (p_fix if skel else p_all)[:, tk, :],
                                        start=(tk == 0), stop=(tk == NKT - 1))
                            if skel:
                                xs_t = xs_fix
                            else:
                             recip_t = recpool.tile([1, 512], F32)
                             nc.vector.reciprocal(recip_t[:], x_ps[0:1, :])
                             # partition-broadcast 1/sum via K=1 outer product
                             r_ps = ops.tile([P, 512], F32, tag="ops")
                             nc.tensor.matmul(r_ps[0 : Dh + 1, :], ones_f32[:],
                                              recip_t[:], start=True, stop=True)
                             r_sb = rsbpool.tile([Dh + 1, 512], F32)
                             nc.scalar.activation(r_sb[:], r_ps[0 : Dh + 1, :],
                                                  mybir.ActivationFunctionType.Copy)
                             xs_t = xspool.tile([Dh + 1, 512], BF16)
                             nc.vector.tensor_mul(xs_t[:], x_ps[:], r_sb[:])
                            out_sb = outpool.tile([P, 4, 512], F32)
                            for m in range(4):
                                out_ps = ops.tile([P, 512], F32, tag="ops")
                                nc.tensor.matmul(out_ps[:],
                                                 xs_t[:, m * P : (m + 1) * P],
                                                 wproj_sb[:],
                                                 start=True, stop=True)
                                nc.vector.tensor_copy(out_sb[:, m, :],
                                                      out_ps[:])
                            row0 = b * S + qb * 512
                            nc.sync.dma_start(
                                part[row0 : row0 + 512, :].rearrange(
                                    "(m p) d -> p m d", p=P),
                                out_sb[:])

    nc.compile()
    return nc


_CACHE = {}


def _get_program(eb: int):
    key = ("prog", eb)
    if key not in _CACHE:
        _CACHE[key] = build_program(eb)
    return _CACHE[key]


def prepare_inputs(hidden_states, attention_mask, alibi_bias, W_qkv, b_qkv,
                   W_proj, b_proj):
    """Host-side prep: transposes, scale folding, exp(alibi), bf16 casts.
    Returns (in_maps, eb)."""
    hidden_states = np.asarray(hidden_states, dtype=np.float32)
    attention_mask = np.asarray(attention_mask)
    alibi_bias = np.asarray(alibi_bias, dtype=np.float32)
    W_qkv = np.asarray(W_qkv, dtype=np.float32)
    b_qkv = np.asarray(b_qkv, dtype=np.float32)
    W_proj = np.asarray(W_proj, dtype=np.float32)
    b_proj = np.asarray(b_proj, dtype=np.float32)

    # per-side scale: total scale SCALE, halved once more to undo the
    # duplicated-K (x2) trick in the score matmul
    s_side = np.float32(np.sqrt(SCALE / 2.0))

    hiddenT = np.ascontiguousarray(
        hidden_states.reshape(BS, D).T).astype(NP_BF16)

    mask_trivial = bool(attention_mask.all())
    eb = 1 if mask_trivial else B

    def ea_layout(eaT):
        # eaT [S(k), S(q)] -> [NQB, 128, NKT, 512] contiguous per qb slice
        return np.ascontiguousarray(
            eaT.reshape(NKT, P, NQB, 512).transpose(2, 1, 0, 3))

    ea_all = []
    for h in range(H):
        eaT = np.exp(alibi_bias[0, h].T).astype(NP_BF16)  # [S(k), S(q)]
        if mask_trivial:
            ea_all.append(ea_layout(eaT)[None])
        else:
            me = np.where(attention_mask, 1.0, 0.0).astype(NP_BF16)  # [B, S]
            ea_all.append(np.stack(
                [ea_layout(eaT * me[bi][:, None]) for bi in range(B)]))
    in_maps = []
    for h in range(H):
        # reference reshapes qkv to (B, S, H, 3*Dh) then splits: head h's
        # q/k/v live in columns [h*3*Dh, h*3*Dh + 3*Dh)
        qs = slice(h * 3 * Dh, h * 3 * Dh + Dh)
        ks = slice(h * 3 * Dh + Dh, h * 3 * Dh + 2 * Dh)
        vs = slice(h * 3 * Dh + 2 * Dh, h * 3 * Dh + 3 * Dh)
        wqk = np.concatenate([W_qkv[:, qs], W_qkv[:, ks]], axis=1) * s_side
        bqk = np.concatenate([b_qkv[qs], b_qkv[ks]]) * s_side
        wv = W_qkv[:, vs]
        bv = b_qkv[vs]
        wproj_aug = np.concatenate(
            [(b_proj if h == 0 else np.zeros_like(b_proj))[None, :],
             W_proj[h * Dh : (h + 1) * Dh, :]], axis=0)
        in_maps.append({
            "hiddenT": hiddenT,
            "ea": ea_all[h],
            "wqk": np.ascontiguousarray(
                wqk.reshape(4, P, P).astype(NP_BF16)),
            "bqk": np.ascontiguousarray(bqk[:, None]),
            "wv": np.ascontiguousarray(wv.reshape(4, P, Dh).astype(NP_BF16)),
            "bv": np.ascontiguousarray(bv[:, None]),
            "wproj": wproj_aug.astype(NP_BF16),
        })
    return in_maps, eb


def kernel(**inputs):
    in_maps, eb = prepare_inputs(**inputs)
    nc = _get_program(eb)
    res = run_bass_kernel_spmd(nc, in_maps, list(range(H)))
    out = res.results[0]["part"].astype(np.float32)
    for h in range(1, H):
        out = out + res.results[h]["part"]
    return out.reshape(B, S, D)



# revision 2
# speedup vs baseline: 1.1023x; 1.1023x over previous
"""Trainium2 Bass kernel for nn_AltAttention (dense transformer attention block).

Reference computation (B=4, S=2048, D=512, H=8, Dh=64):
    qkv  = hidden @ W_qkv + b_qkv                      -> q, k, v per head
    attn = softmax(q k^T * D**-0.5 + alibi, masked)
    out  = (attn @ v) @ W_proj + b_proj
Sharding: one head per NeuronCore; host sums the 8 partial projections.

v2 changes over the baseline:
  - scores use row-packed K=64 matmul pairs (array rows 0:63 / 64:127 via
    base-partition-derived tile_position) so two key tiles stream through
    the PE concurrently; qkT = [q;k] and kqT = [k;q] duplicated layouts
    provide both operands at both base partitions.
  - q and k are evicted from PSUM in one tensor_scalar_add per 1024-wide
    chunk; kqT is built with partition-swapping SBUF DMAs.
  - the exp(s)*exp(alibi) multiply is split between DVE and GPSIMD.
  - the PE stream is software-pipelined: scores run three groups ahead of
    attn@V (five for GPSIMD-multiplied groups), and each pair's normalize +
    proj tail is emitted interleaved into the NEXT pair's groups, so the PE
    and ACT engines never drain at pair boundaries.
  - PSUM pools are shared across phases (qk reuses the scores pool, v^T
    reuses the x pool) so phase 2 does not wait for a pool handover.
  - hidden-state DMAs are split per 512-row chunk to shorten the phase-1
    fill latency; the qb=0 exp(alibi) tile is prefetched during phase 1.
"""

import sys

sys.path.insert(0, "/opt/trn_rl_repo")

import numpy as np
import ml_dtypes

import concourse.bass as bass
import concourse.tile as tile
from concourse import bacc, mybir
from concourse.bass_utils import run_bass_kernel_spmd
from concourse import library_config

BF16 = mybir.dt.bfloat16
F32 = mybir.dt.float32
NP_BF16 = ml_dtypes.bfloat16

B, S, D, H = 4, 2048, 512, 8
Dh = D // H  # 64
BS = B * S  # 8192
P = 128
NKT = S // P  # 16 ks tiles per batch
NQB = S // 512  # 4 query blocks of 512 per batch
NSC = S // 512  # 4 s-chunks of 512 per batch (qkv phase)
SCALE = D ** (-0.5)
NG = NKT // 2  # 8 groups of 2 key tiles per query block

# exp(alibi)-multiply groups handled by GPSIMD instead of DVE (of NG)
GPS_GROUPS = (2, 5)
# add alibi on the PE (identity-matmul accumulation) instead of exp(alibi) mul
ALIBI_PE = False


def build_program(eb: int, repeat: int = 1, phases=(1, 2), skel=False,
                  gps_groups=(2, 4, 6), rsb_gps=True, interleave_p1=True,
                  row_pack=True, p1_act=True, av_lag=4, out_fp16=True,
                  alibi_pe=None):
    if alibi_pe is None:
        alibi_pe = ALIBI_PE
    """Build the per-core Bass program. eb = number of exp-alibi slices
    (1 when the attention mask is all ones, B otherwise)."""
    nc = bacc.Bacc("TRN2", target_bir_lowering=False, debug=False, num_devices=H)

    hiddenT = nc.dram_tensor("hiddenT", [D, BS], BF16, kind="ExternalInput")
    # ea layout: [eb, NQB, 128, NKT, 512] so each (e, qb) slice is one
    # contiguous 2 MB DMA
    ea = nc.dram_tensor("ea", [eb, NQB, P, NKT, 512], BF16,
                        kind="ExternalInput")
    wqk = nc.dram_tensor("wqk", [4, P, P], BF16, kind="ExternalInput")
    bqk = nc.dram_tensor("bqk", [P, 1], F32, kind="ExternalInput")
    wv = nc.dram_tensor("wv", [4, P, Dh], BF16, kind="ExternalInput")
    bv = nc.dram_tensor("bv", [Dh, 1], F32, kind="ExternalInput")
    wproj = nc.dram_tensor("wproj", [Dh + 1, D], BF16, kind="ExternalInput")
    ident = nc.dram_tensor("ident", [P, P], BF16, kind="ExternalInput")
    part = nc.dram_tensor("part", [BS, D],
                          mybir.dt.float16 if out_fp16 else F32,
                          kind="ExternalOutput")

    hT_re = hiddenT[:].rearrange("(c p) s -> p c s", p=P)  # [128, 4, 8192]

    with tile.TileContext(nc) as tc:
        with tc.tile_pool(name="consts", bufs=1) as consts, \
             tc.tile_pool(name="persist", bufs=1) as persist:
            if rsb_gps:
                nc.gpsimd.load_library(library_config.proxy)
            wqk_sb = consts.tile([P, 4, P], BF16)
            nc.sync.dma_start(wqk_sb[:], wqk[:].rearrange("c p m -> p c m"))
            wv_sb = consts.tile([P, 4, Dh], BF16)
            nc.sync.dma_start(wv_sb[:], wv[:].rearrange("c p m -> p c m"))
            bqk_sb = consts.tile([P, 1], F32)
            nc.sync.dma_start(bqk_sb[:], bqk[:])
            bv_sb = consts.tile([Dh, 1], F32)
            nc.sync.dma_start(bv_sb[:], bv[:])
            wproj_sb = consts.tile([Dh + 1, D], BF16)
            nc.sync.dma_start(wproj_sb[:], wproj[:])
            ones_f32 = consts.tile([1, Dh + 1], F32)
            nc.vector.memset(ones_f32[:], 1.0)
            ident_sb = consts.tile([P, P], BF16)
            nc.sync.dma_start(ident_sb[:], ident[:])

            # qkT rows 0:64 = q, rows 64:128 = k; kqT is the partition swap
            qkT = persist.tile([P, BS], BF16)
            kqT = persist.tile([P, BS], BF16)
            # padded layout: tile t = [:, t, 63:128]; col 63 = ones (sums
            # row), cols 64:128 = v^T (DMA-transpose needs 128B-aligned
            # dest offsets)
            vaug = persist.tile([P, B * NKT, P], BF16)
            nc.vector.memset(vaug[:, :, Dh - 1 : Dh], 1.0)
            vt_all = persist.tile([Dh, B * NSC, 512], BF16)

            if skel:
                p_fix = persist.tile([P, NKT, 512], BF16)
                nc.vector.memset(p_fix[:], 0.01)
                xs_fix = persist.tile([Dh + 1, 512], BF16)
                nc.vector.memset(xs_fix[:], 0.02)
            if 1 not in phases:
                nc.vector.memset(qkT[:], 0.01)
                nc.vector.memset(kqT[:], 0.01)
                nc.vector.memset(vaug[:], 0.01)
                nc.vector.memset(vaug[:, :, Dh - 1 : Dh], 1.0)

            for rep in range(repeat):
              with tc.tile_pool(name="eapool", bufs=2) as eapool, \
                   tc.tile_pool(name="hpool", bufs=2) as hpool, \
                   tc.tile_pool(name="ppool", bufs=2) as ppool, \
                   tc.tile_pool(name="xspool", bufs=2) as xspool, \
                   tc.tile_pool(name="rsbpool", bufs=2) as rsbpool, \
                   tc.tile_pool(name="recpool", bufs=2) as recpool, \
                   tc.tile_pool(name="outpool", bufs=2) as outpool, \
                   tc.tile_pool(name="spool", bufs=2, space="PSUM") as spool, \
                   tc.tile_pool(name="vxpool", bufs=2, space="PSUM") as vxpool, \
                   tc.tile_pool(name="qops", bufs=2, space="PSUM") as qops:
                ea_prefetch = [None]
                ht_box = {}

                def p1_quanta(b):
                    """Phase-1 work for one batch as a list of emission
                    closures (interleavable into phase-2 pairs)."""
                    steps = []

                    def q_load(b=b):
                        ht = hpool.tile([P, 4, S], BF16, name="ht", tag="ht")
                        ht_box[b] = ht
                        nc.sync.dma_start(
                            ht[:], hT_re[:, :, b * S : (b + 1) * S])
                        if b == 0 and 2 in phases and eb == 1:
                            # prefetch qb=0 exp(alibi) during phase 1
                            ea_prefetch[0] = eapool.tile(
                                [P, NKT, 512], BF16, name="ea_t", tag="ea")
                            nc.sync.dma_start(ea_prefetch[0][:], ea[0, 0])
                    steps.append(q_load)

                    def mk_sci(b, sci):
                        def q_sci():
                            ht = ht_box[b]
                            ssl = slice(sci * 512, (sci + 1) * 512)
                            qk_ps = qops.tile([P, 512], F32, name="qk_ps",
                                              tag="qo")
                            for c in range(4):
                                nc.tensor.matmul(qk_ps[:], wqk_sb[:, c, :],
                                                 ht[:, c, ssl],
                                                 start=(c == 0), stop=(c == 3))
                            col0 = b * S + sci * 512
                            if p1_act:
                                nc.scalar.activation(
                                    qkT[:, col0 : col0 + 512], qk_ps[:],
                                    mybir.ActivationFunctionType.Identity,
                                    bias=bqk_sb[:])
                            else:
                                nc.vector.tensor_scalar_add(
                                    qkT[:, col0 : col0 + 512], qk_ps[:],
                                    bqk_sb[:])
                            vt_ps = vxpool.tile([Dh + 1, 512], F32,
                                                name="vt_ps", tag="vx")
                            for c in range(4):
                                nc.tensor.matmul(vt_ps[0:Dh, :],
                                                 wv_sb[:, c, :], ht[:, c, ssl],
                                                 start=(c == 0), stop=(c == 3))
                            i = b * NSC + sci
                            if p1_act:
                                nc.scalar.activation(
                                    vt_all[:, i, :], vt_ps[0:Dh, :],
                                    mybir.ActivationFunctionType.Identity,
                                    bias=bv_sb[:])
                            else:
                                nc.vector.tensor_scalar_add(
                                    vt_all[:, i, :], vt_ps[0:Dh, :], bv_sb[:])
                        return q_sci
                    steps += [mk_sci(b, sci) for sci in range(NSC)]

                    def q_post(b=b):
                        bsl = slice(b * S, (b + 1) * S)
                        # partition-swap copy: kqT = [k; q]
                        nc.sync.dma_start(kqT[0:Dh, bsl], qkT[Dh:P, bsl])
                        nc.sync.dma_start(kqT[Dh:P, bsl], qkT[0:Dh, bsl])
                        # v transpose for this batch (one XBAR-mode DMA)
                        nc.sync.dma_start(
                            vaug[:, b * NKT : (b + 1) * NKT, Dh:P],
                            vt_all[:, b * NSC : (b + 1) * NSC, :]
                            .rearrange("p a b -> p (a b)"),
                            transpose=True)
                    steps.append(q_post)
                    return steps

                # ------------- phase 2: attention + proj -------------
                if 2 in phases:
                    if 1 in phases:
                        for st in p1_quanta(0):
                            st()
                    prev_tail = []
                    p1_pend = []

                    def make_tail(x_ps, row0):
                        """Normalize + proj for one finished pair, as
                        emission closures interleaved into the next pair."""
                        steps = []
                        if skel:
                            xs_box = [xs_fix]
                        else:
                            xs_box = [None]
                            rec_box = [None]
                            rps_box = [None]
                            rsb_box = [None]

                            def t_recip():
                                rec_box[0] = recpool.tile([1, 512], F32,
                                                          name="recip_t")
                                nc.vector.reciprocal(rec_box[0][:],
                                                     x_ps[0:1, :])

                            def t_rbc():
                                rsb_box[0] = rsbpool.tile([Dh + 1, 512], F32,
                                                          name="r_sb")
                                if rsb_gps:
                                    nc.gpsimd.partition_broadcast(
                                        rsb_box[0][:], rec_box[0][:],
                                        channels=Dh + 1)
                                else:
                                    rps_box[0] = qops.tile([P, 512], F32,
                                                           name="r_ps",
                                                           tag="qo")
                                    nc.tensor.matmul(
                                        rps_box[0][0 : Dh + 1, :],
                                        ones_f32[:], rec_box[0][:],
                                        start=True, stop=True)
                                    nc.scalar.activation(
                                        rsb_box[0][:],
                                        rps_box[0][0 : Dh + 1, :],
                                        mybir.ActivationFunctionType.Copy)

                            def t_xs():
                                xs_box[0] = xspool.tile([Dh + 1, 512], BF16,
                                                        name="xs_t")
                                nc.vector.tensor_mul(xs_box[0][:], x_ps[:],
                                                     rsb_box[0][:])

                            steps += [t_recip, t_rbc, t_xs]
                        out_box = [None]

                        def mk_proj(m):
                            def t_proj():
                                if out_box[0] is None:
                                    out_box[0] = outpool.tile(
                                        [P, 4, 512],
                                        mybir.dt.float16 if out_fp16 else F32,
                                        name="out_sb")
                                out_ps = qops.tile([P, 512], F32,
                                                   name="out_ps", tag="qo")
                                nc.tensor.matmul(
                                    out_ps[:],
                                    xs_box[0][:, m * P : (m + 1) * P],
                                    wproj_sb[:], start=True, stop=True)
                                nc.vector.tensor_copy(out_box[0][:, m, :],
                                                      out_ps[:])
                                if m == 3:
                                    nc.sync.dma_start(
                                        part[row0 : row0 + 512, :].rearrange(
                                            "(m p) d -> p m d", p=P),
                                        out_box[0][:])
                            return t_proj

                        steps += [mk_proj(m) for m in range(4)]
                        return steps

                    for qb in range(NQB):
                        if eb == 1:
                            if qb == 0 and ea_prefetch[0] is not None:
                                ea_t = ea_prefetch[0]
                            else:
                                ea_t = eapool.tile([P, NKT, 512], BF16,
                                                   tag="ea", name="ea_t")
                                nc.sync.dma_start(ea_t[:], ea[0, qb])
                        for b in range(B):
                            if eb != 1:
                                ea_t = eapool.tile([P, NKT, 512], BF16,
                                                   tag="ea", name="ea_t")
                                nc.sync.dma_start(ea_t[:], ea[b, qb])
                            qsl = slice(b * S + qb * 512,
                                        b * S + (qb + 1) * 512)
                            x_ps = vxpool.tile([Dh + 1, 512], F32,
                                               name="x_ps", tag="vx")
                            p_all = ppool.tile([P, NKT, 512], BF16,
                                               name="p_all")
                            if (1 in phases and interleave_p1
                                    and qb == 0 and b + 1 < B):
                                p1_pend.extend(p1_quanta(b + 1))

                            def scores(g):
                                s_ps = spool.tile([P, 1024], F32,
                                                  name="s_ps", tag="s")
                                # row-packed pair: two K=64 matmuls running
                                # concurrently on rows 0:63 / 64:127
                                tkA, tkB = g * 2, g * 2 + 1
                                kslA = slice(b * S + tkA * P,
                                             b * S + (tkA + 1) * P)
                                kslB = slice(b * S + tkB * P,
                                             b * S + (tkB + 1) * P)
                                if row_pack:
                                    nc.tensor.matmul(
                                        s_ps[:, 0:512],
                                        kqT[0:Dh, kslA], qkT[0:Dh, qsl],
                                        start=True, stop=not alibi_pe)
                                    nc.tensor.matmul(
                                        s_ps[:, 512:1024],
                                        qkT[Dh:P, kslB], kqT[Dh:P, qsl],
                                        start=True, stop=not alibi_pe)
                                    if alibi_pe:
                                        # s += alibi via identity-weight
                                        # accumulating matmuls
                                        asl = ea_t[:, 2 * g : 2 * g + 2, :]
                                        nc.tensor.matmul(
                                            s_ps[:, 0:512], ident_sb[:],
                                            asl[:, 0, :],
                                            start=False, stop=True)
                                        nc.tensor.matmul(
                                            s_ps[:, 512:1024], ident_sb[:],
                                            asl[:, 1, :],
                                            start=False, stop=True)
                                else:
                                    # unpacked: K=128 over [q;q]/[k;k]-style
                                    # dup contraction is unavailable here, so
                                    # use K=64 serial matmuls at base 0
                                    nc.tensor.matmul(
                                        s_ps[:, 0:512],
                                        kqT[0:Dh, kslA], qkT[0:Dh, qsl],
                                        start=True, stop=True)
                                    nc.tensor.matmul(
                                        s_ps[:, 512:1024],
                                        kqT[0:Dh, kslB], qkT[0:Dh, qsl],
                                        start=True, stop=True)
                                return s_ps

                            def softmax_mul(g, s_ps):
                                psl = p_all[:, 2 * g : 2 * g + 2, :]\
                                    .rearrange("p a b -> p (a b)")
                                easl = ea_t[:, 2 * g : 2 * g + 2, :]\
                                    .rearrange("p a b -> p (a b)")
                                if not skel:
                                    nc.scalar.activation(
                                        psl, s_ps[:],
                                        mybir.ActivationFunctionType.Exp)
                                    if not alibi_pe:
                                        eng = (nc.gpsimd if g in gps_groups
                                               else nc.vector)
                                        eng.tensor_mul(psl, psl, easl)

                            av_n = [0]

                            def attnv(g):
                                for j in range(2):
                                    tk = g * 2 + j
                                    t = b * NKT + tk
                                    i = av_n[0]
                                    av_n[0] += 1
                                    nc.tensor.matmul(
                                        x_ps[:], vaug[:, t, Dh - 1 : P],
                                        (p_fix if skel else p_all)[:, tk, :],
                                        start=(i == 0), stop=(i == NKT - 1))

                            pend = []
                            for g in range(NG):
                                s_ps = scores(g)
                                softmax_mul(g, s_ps)
                                if prev_tail:
                                    prev_tail.pop(0)()
                                if p1_pend:
                                    p1_pend.pop(0)()
                                lag = av_lag + 2 if g in gps_groups else av_lag
                                pend.append((g + lag, g))
                                for slot, pg in list(pend):
                                    if slot <= g + 1:
                                        attnv(pg)
                                        pend.remove((slot, pg))
                            for slot, pg in sorted(pend):
                                attnv(pg)
                            while prev_tail:
                                prev_tail.pop(0)()
                            while p1_pend:
                                p1_pend.pop(0)()

                            prev_tail = make_tail(x_ps, b * S + qb * 512)
                    while prev_tail:
                        prev_tail.pop(0)()
                elif 1 in phases:
                    for b in range(B):
                        for st in p1_quanta(b):
                            st()

    nc.compile()
    return nc


_CACHE = {}


def _get_program(eb: int):
    key = ("prog", eb)
    if key not in _CACHE:
        _CACHE[key] = build_program(eb)
    return _CACHE[key]


def prepare_inputs(hidden_states, attention_mask, alibi_bias, W_qkv, b_qkv,
                   W_proj, b_proj):
    """Host-side prep: transposes, scale folding, exp(alibi), bf16 casts.
    Returns (in_maps, eb)."""
    hidden_states = np.asarray(hidden_states, dtype=np.float32)
    attention_mask = np.asarray(attention_mask)
    alibi_bias = np.asarray(alibi_bias, dtype=np.float32)
    W_qkv = np.asarray(W_qkv, dtype=np.float32)
    b_qkv = np.asarray(b_qkv, dtype=np.float32)
    W_proj = np.asarray(W_proj, dtype=np.float32)
    b_proj = np.asarray(b_proj, dtype=np.float32)

    # per-side scale (K=64 row-packed scores: no doubled-sum compensation)
    s_side = np.float32(np.sqrt(SCALE))

    hiddenT = np.ascontiguousarray(
        hidden_states.reshape(BS, D).T).astype(NP_BF16)

    mask_trivial = bool(attention_mask.all())
    eb = 1 if mask_trivial else B

    def ea_layout(eaT):
        # eaT [S(k), S(q)] -> [NQB, 128, NKT, 512] contiguous per qb slice
        return np.ascontiguousarray(
            eaT.reshape(NKT, P, NQB, 512).transpose(2, 1, 0, 3))

    ea_all = []
    for h in range(H):
        aT = alibi_bias[0, h].T  # [S(k), S(q)]
        if ALIBI_PE:
            eaT = aT.astype(NP_BF16)
            if mask_trivial:
                ea_all.append(ea_layout(eaT)[None])
            else:
                mb = np.where(attention_mask, 0.0, -30000.0)  # [B, S]
                ea_all.append(np.stack(
                    [ea_layout((aT + mb[bi][:, None]).astype(NP_BF16))
                     for bi in range(B)]))
        else:
            eaT = np.exp(aT).astype(NP_BF16)
            if mask_trivial:
                ea_all.append(ea_layout(eaT)[None])
            else:
                me = np.where(attention_mask, 1.0, 0.0).astype(NP_BF16)
                ea_all.append(np.stack(
                    [ea_layout(eaT * me[bi][:, None]) for bi in range(B)]))
    in_maps = []
    for h in range(H):
        # reference reshapes qkv to (B, S, H, 3*Dh) then splits: head h's
        # q/k/v live in columns [h*3*Dh, h*3*Dh + 3*Dh)
        qs = slice(h * 3 * Dh, h * 3 * Dh + Dh)
        ks = slice(h * 3 * Dh + Dh, h * 3 * Dh + 2 * Dh)
        vs = slice(h * 3 * Dh + 2 * Dh, h * 3 * Dh + 3 * Dh)
        wqk = np.concatenate([W_qkv[:, qs], W_qkv[:, ks]], axis=1) * s_side
        bqk = np.concatenate([b_qkv[qs], b_qkv[ks]]) * s_side
        wv = W_qkv[:, vs]
        bv = b_qkv[vs]
        wproj_aug = np.concatenate(
            [(b_proj if h == 0 else np.zeros_like(b_proj))[None, :],
             W_proj[h * Dh : (h + 1) * Dh, :]], axis=0)
        in_maps.append({
            "hiddenT": hiddenT,
            "ea": ea_all[h],
            "ident": np.eye(P, dtype=NP_BF16),
            "wqk": np.ascontiguousarray(
                wqk.reshape(4, P, P).astype(NP_BF16)),
            "bqk": np.ascontiguousarray(bqk[:, None]),
            "wv": np.ascontiguousarray(wv.reshape(4, P, Dh).astype(NP_BF16)),
            "bv": np.ascontiguousarray(bv[:, None]),
            "wproj": wproj_aug.astype(NP_BF16),
        })
    return in_maps, eb


def kernel(**inputs):
    in_maps, eb = prepare_inputs(**inputs)
    nc = _get_program(eb)
    res = run_bass_kernel_spmd(nc, in_maps, list(range(H)))
    out = res.results[0]["part"].astype(np.float32)
    for h in range(1, H):
        out = out + res.results[h]["part"]
    return out.reshape(B, S, D)
